# revision 6
# baseline (speedup 1.0000x reference)
"""GraphSAGE (2-layer, mean aggregation) on 8 Trainium2 NeuronCores — v3.

v3 over v2:
  - The layer-1 -> layer-2 handoff is pipelined: h rows are stored per-block
    into two DRAM slabs (local rows < 3125 and >= 3125), AllGathered as two
    Shared-output collectives.  Layer-2 edges are split by
    (src mod 6250) < 3125 so its "lo2" gather stream only needs the first
    collective, which completes while layer-1 is still draining.
  - Layer-2 aggregation is two-pass (lo2 partial sums evacuated to SBUF,
    hi2 added on top) so lo2 tiles release immediately.
  - Startup loads reordered (idx/cnt first).
"""

import math
from contextlib import ExitStack

import numpy as np
import ml_dtypes

import concourse.bass as bass
import concourse.bacc as bacc
import concourse.mybir as mybir
import concourse.tile as tile
from concourse import bass_utils

P = 128
N_NODES = 50000
N_EDGES = 800000
D_IN = 128
D_HID = 128
D_OUT = 40
N_CORES = 8
SPLIT = 25000             # layer-1 src-table split (int16 gather index limit)
HALFR = 3125              # per-core local-row split for the chunked collective
SP = False

BF16 = ml_dtypes.bfloat16


def _wrap_idxs(idx_flat):
    n = idx_flat.shape[0]
    assert n % 16 == 0
    w = idx_flat.reshape(n // 16, 16).T.astype(np.int16)
    return np.tile(w, (8, 1))


def preprocess(edge_index, n_nodes=N_NODES, n_cores=N_CORES, split=SPLIT):
    src = np.asarray(edge_index[0], dtype=np.int64)
    dst = np.asarray(edge_index[1], dtype=np.int64)
    counts = np.bincount(dst, minlength=n_nodes)
    inv_deg = (1.0 / np.maximum(counts, 1)).astype(np.float32)

    rows_per = n_nodes // n_cores
    nblk = math.ceil(rows_per / P)

    order = np.argsort(dst, kind="stable")
    s_s, d_s = src[order], dst[order]

    # Layer-2 remapped index: node n lives in hfirst (sel2=0) at
    # (n//rows_per)*HALFR + (n%rows_per), or in hsecond at that - HALFR.
    def l2_map(s):
        q, r = s // rows_per, s % rows_per
        half = (r >= HALFR).astype(np.int64)
        return half, q * HALFR + r - HALFR * half

    blk_edges = {}
    maxima = [0, 0, 0, 0]   # lo1, hi1, lo2, hi2
    for k in range(n_cores):
        base = k * rows_per
        for b in range(nblk):
            r0 = base + b * P
            r1 = min(base + rows_per, r0 + P)
            e0 = np.searchsorted(d_s, r0, side="left")
            e1 = np.searchsorted(d_s, r1, side="left")
            s_seg, d_seg = s_s[e0:e1], d_s[e0:e1]
            half2, idx2 = l2_map(s_seg)
            sels = [s_seg < split, s_seg >= split, half2 == 0, half2 == 1]
            idxs = [s_seg, s_seg - split, idx2, idx2]
            blk_edges[(k, b)] = (d_seg - r0, inv_deg[d_seg], sels, idxs)
            for t in range(4):
                # one gather slot per unique src in the segment
                maxima[t] = max(maxima[t], len(np.unique(idxs[t][sels[t]])))

    Ls = [max(1, math.ceil(mx / P)) for mx in maxima]
    Cs = [nblk * L for L in Ls]

    per_core = []
    for k in range(n_cores):
        idx_a = [np.full((C, P), -1, np.int16) for C in Cs]
        ent = [[] for _ in range(4)]   # (chunk, part, dstcol, val) per stream
        cnt = np.ones((2, nblk, 2), np.int32)
        for b in range(nblk):
            dd_all, vv_all, sels, idxs = blk_edges[(k, b)]
            for t in range(4):
                sel = sels[t]
                ss = idxs[t][sel]
                dd = dd_all[sel]
                vv = vv_all[sel]
                uniq, inv = np.unique(ss, return_inverse=True)
                n = uniq.shape[0]
                cnt[t // 2, b, t % 2] = max(n, 1)
                L = Ls[t]
                c0 = b * L
                fl_i = idx_a[t][c0 : c0 + L].reshape(-1)
                fl_i[:n] = uniq.astype(np.int16)
                if n == 0:
                    fl_i[0] = 0
                # edge e contributes val at gather slot inv[e], dst col dd[e]
                ent[t].append((c0 + inv // P, inv % P, dd, vv))

        def onehot(entries, C):
            o = np.zeros((C, P, P), np.float32)
            cc = np.concatenate([e[0] for e in entries])
            pp = np.concatenate([e[1] for e in entries])
            dd = np.concatenate([e[2] for e in entries]).astype(np.int64)
            vv = np.concatenate([e[3] for e in entries])
            np.add.at(o, (cc, pp, dd), vv)
            return np.ascontiguousarray(
                o.transpose(1, 0, 2).reshape(P, C * P).astype(BF16))

        per_core.append(
            dict(
                idx=[_wrap_idxs(a.reshape(-1)) for a in idx_a],
                oh=[onehot(ent[t], Cs[t]) for t in range(4)],
                cnt=np.ascontiguousarray(cnt.reshape(1, 4 * nblk)),
            )
        )

    meta = dict(
        n_nodes=n_nodes, n_cores=n_cores, rows_per=rows_per, nblk=nblk,
        Ls=tuple(Ls), Cs=tuple(Cs), split=split,
    )
    return meta, per_core


def build_graph(nc, m, d_in=D_IN, d_out=D_OUT, debug_skip=()):
    dt = mybir.dt
    alu = mybir.AluOpType
    act = mybir.ActivationFunctionType
    n_nodes, rows_per, nblk = m["n_nodes"], m["rows_per"], m["nblk"]
    Ls, Cs = m["Ls"], m["Cs"]
    split = m["split"]

    x_all = nc.dram_tensor("x_all", [n_nodes, d_in], dt.bfloat16, kind="ExternalInput")
    xT_d = nc.dram_tensor("xT", [P, rows_per], dt.bfloat16, kind="ExternalInput")
    idx_d = [nc.dram_tensor(f"idx{t}", [P, Cs[t] * 8], dt.int16, kind="ExternalInput")
             for t in range(4)]
    oh_d = [nc.dram_tensor(f"oh{t}", [P, Cs[t] * P], dt.bfloat16, kind="ExternalInput")
            for t in range(4)]
    w1l_d = nc.dram_tensor("w1lT", [P, d_in], dt.bfloat16, kind="ExternalInput")
    w1r_d = nc.dram_tensor("w1rT", [P, d_in], dt.bfloat16, kind="ExternalInput")
    w2l_d = nc.dram_tensor("w2lT", [P, d_out], dt.bfloat16, kind="ExternalInput")
    w2r_d = nc.dram_tensor("w2rT", [P, d_out], dt.bfloat16, kind="ExternalInput")
    b1_d = nc.dram_tensor("b1r", [1, d_in], dt.bfloat16, kind="ExternalInput")
    b2_d = nc.dram_tensor("b2r", [1, d_out], dt.bfloat16, kind="ExternalInput")
    cnt_d = nc.dram_tensor("cnt", [1, 4 * nblk], dt.int32, kind="ExternalInput")
    out_d = nc.dram_tensor("out", [rows_per, d_out], dt.float32, kind="ExternalOutput")

    with tile.TileContext(nc) as tc, ExitStack() as ctx:
        sb = ctx.enter_context(tc.tile_pool(name="sb", bufs=1))
        dram = ctx.enter_context(tc.tile_pool(name="dram", bufs=1, space="DRAM"))
        psum = ctx.enter_context(tc.tile_pool(name="psum", bufs=8, space="PSUM"))
        g_p = [ctx.enter_context(tc.tile_pool(name=f"g{t}", bufs=6))
               for t in range(4)]
        o_p = [ctx.enter_context(tc.tile_pool(name=f"o{t}", bufs=4))
               for t in range(4)]
        st_p = ctx.enter_context(tc.tile_pool(name="st", bufs=2))

        from concourse import library_config
        nc.gpsimd.load_library(library_config.mlp)

        wz = sb.tile([P, 8], mybir.dt.int16, name="wz")
        nc.vector.memset(wz[:], 0.0)

        def load(shape, dtype, src, name):
            t = sb.tile(shape, dtype, name=name)
            nc.sync.dma_start(t[:], src[:])
            return t

        # Load order matters: the first gather only needs cnt + the first few
        # blocks of idx0/idx1, so those index tables are loaded in two pieces
        # (a single [128, C*8] DMA is 128 serial descriptors ~ 17us).
        cnt_sb = load([1, 4 * nblk], dt.int32, cnt_d.ap(), "cnt_sb")
        idx_sb = [None] * 4
        for t in (0, 1):
            idx_sb[t] = load([P, Cs[t] * 8], dt.int16, idx_d[t].ap(), f"idx{t}_sb")
        xT_sb = load([P, rows_per], dt.bfloat16, xT_d.ap(), "xT_sb")
        for t in (2, 3):
            idx_sb[t] = load([P, Cs[t] * 8], dt.int16, idx_d[t].ap(), f"idx{t}_sb")
        w1l_sb = load([P, d_in], dt.bfloat16, w1l_d.ap(), "w1l_sb")
        w1r_sb = load([P, d_in], dt.bfloat16, w1r_d.ap(), "w1r_sb")
        w2l_sb = load([P, d_out], dt.bfloat16, w2l_d.ap(), "w2l_sb")
        w2r_sb = load([P, d_out], dt.bfloat16, w2r_d.ap(), "w2r_sb")
        b1_sb = load([1, d_in], dt.bfloat16, b1_d.ap(), "b1_sb")
        b2_sb = load([1, d_out], dt.bfloat16, b2_d.ap(), "b2_sb")
        creg = nc.gpsimd.alloc_register("gcnt")

        ones_sb = sb.tile([1, 512], dt.bfloat16, name="ones_sb")
        nc.vector.memset(ones_sb[:], 1.0)

        meanT = sb.tile([P, rows_per], dt.bfloat16, name="meanT")
        meanhT = sb.tile([P, rows_per], dt.bfloat16, name="meanhT")
        hT = sb.tile([P, rows_per], dt.bfloat16, name="hT")

        hshA = dram.tile([HALFR, d_in], dt.bfloat16, name="hshA")
        hshB = dram.tile([HALFR, d_in], dt.bfloat16, name="hshB")
        hfirst = dram.tile([n_nodes // 2, d_in], dt.bfloat16, name="hfirst",
                           addr_space="Shared")
        hsecond = dram.tile([n_nodes // 2, d_in], dt.bfloat16, name="hsecond",
                            addr_space="Shared")

        # Pre-zero every physical gather buffer (pad slots skipped by
        # num_idxs_reg must stay finite for the 0-weighted matmul columns).
        for t in range(4):
            for _ in range(6):
                z = g_p[t].tile([P, Ls[t], P], dt.bfloat16, tag=f"g{t}",
                                name=f"z{t}")
                nc.vector.memset(z[:], 0.0)

        qctr = [0]

        wouts = [sb.tile([P, 1, P], dt.bfloat16, name=f"wo{q}") for q in range(4)]
        for q in range(4):
            nc.gpsimd.dma_gather(wouts[q][:], x_all.ap()[0:split, :], wz[:],
                                 P, P, d_in, elem_step=d_in, single_packet=SP,
                                 queue_num=q)

        def issue_gather(t, b, src_ap):
            L = s_L = Ls[t]
            gt = g_p[t].tile([P, L, P], dt.bfloat16, tag=f"g{t}", name=f"g{t}")
            if "gather" in debug_skip:
                nc.vector.memset(gt[:], 0.0)
            else:
                nc.gpsimd.reg_load(creg, cnt_sb[0:1, (t // 2) * 2 * nblk
                                                + 2 * b + (t % 2) :
                                                (t // 2) * 2 * nblk
                                                + 2 * b + (t % 2) + 1])
                nc.gpsimd.dma_gather(
                    gt[:], src_ap,
                    idx_sb[t][:, b * L * 8 : (b + 1) * L * 8],
                    L * P, creg, d_in, elem_step=d_in, single_packet=SP,
                    queue_num=qctr[0] % nc.num_swdge_queues,
                )
                qctr[0] += 1
            ot = o_p[t].tile([P, L, P], dt.bfloat16, tag=f"o{t}", name=f"o{t}")
            nc.sync.dma_start(ot[:], oh_d[t].ap()[:, b * L * P : (b + 1) * L * P])
            return gt, ot

        AHEAD = 3

        # ============ layer 1 ============
        l1_src = [x_all.ap()[0:split, :], x_all.ap()[split:n_nodes, :]]
        tiles = {}

        def l1_prefetch(b):
            for bp in range(b, min(b + AHEAD, nblk)):
                for t in (0, 1):
                    if (bp, t) not in tiles:
                        tiles[(bp, t)] = issue_gather(t, bp, l1_src[t])

        def store_h(hrow, c0, bs):
            """Store h rows [c0, c0+bs) into hshA/hshB split at HALFR."""
            if c0 + bs <= HALFR:
                nc.scalar.dma_start(hshA[c0 : c0 + bs, :], hrow[:bs, :])
            elif c0 >= HALFR:
                nc.scalar.dma_start(hshB[c0 - HALFR : c0 - HALFR + bs, :],
                                    hrow[:bs, :])
            else:
                cut = HALFR - c0
                nc.scalar.dma_start(hshA[c0:HALFR, :], hrow[:cut, :])
                nc.scalar.dma_start(hshB[0 : bs - cut, :], hrow[cut:bs, :])

        for b in range(nblk):
            l1_prefetch(b)
            c0 = b * P
            bs = min(P, rows_per - c0)
            ps = psum.tile([P, 512], dt.float32, tag="ps", name="ps_agg")
            nops = Ls[0] + Ls[1]
            i = 0
            for t in (0, 1):
                gt, ot = tiles.pop((b, t))
                for j in range(Ls[t]):
                    nc.tensor.matmul(ps[:, :P], lhsT=gt[:, j, :], rhs=ot[:, j, :],
                                     start=(i == 0), stop=(i == nops - 1))
                    i += 1
            nc.vector.tensor_copy(meanT[:, c0 : c0 + bs], ps[:, :bs])

            # row-dense: h block -> hshA/hshB immediately (feeds collectives)
            ps2 = psum.tile([P, 512], dt.float32, tag="ps", name="ps_r")
            nc.tensor.matmul(ps2[:bs, :d_in], lhsT=meanT[:, c0 : c0 + bs],
                             rhs=w1l_sb[:], start=True, stop=False)
            nc.tensor.matmul(ps2[:bs, :d_in], lhsT=xT_sb[:, c0 : c0 + bs],
                             rhs=w1r_sb[:], start=False, stop=False)
            nc.tensor.matmul(ps2[:bs, :d_in], lhsT=ones_sb[:, :bs], rhs=b1_sb[:],
                             start=False, stop=True)
            hrow = st_p.tile([P, d_in], dt.bfloat16, tag="st", name="hrow")
            nc.scalar.activation(hrow[:bs, :], ps2[:bs, :d_in], act.Relu)
            store_h(hrow, c0, bs)

            if b == 25 and "collective" not in debug_skip:
                nc.gpsimd.collective_compute(
                    "AllGather", alu.bypass,
                    replica_groups=[list(range(m["n_cores"]))],
                    ins=[hshA[:].opt()], outs=[hfirst[:].opt()],
                )

        if "collective" in debug_skip:
            nc.sync.dma_start(hfirst[0:HALFR, :], hshA[:])
            nc.sync.dma_start(hsecond[0:HALFR, :], hshB[:])

        # column-dense for hT (used by the layer-2 output dense, needed late)
        for c0 in range(0, rows_per, 512):
            w = min(512, rows_per - c0)
            ps = psum.tile([P, 512], dt.float32, tag="ps", name="ps_d")
            nc.tensor.matmul(ps[:, :w], lhsT=w1l_sb[:], rhs=meanT[:, c0 : c0 + w],
                             start=True, stop=False)
            nc.tensor.matmul(ps[:, :w], lhsT=w1r_sb[:], rhs=xT_sb[:, c0 : c0 + w],
                             start=False, stop=False)
            nc.tensor.matmul(ps[:, :w], lhsT=b1_sb[:], rhs=ones_sb[:, :w],
                             start=False, stop=True)
            nc.scalar.activation(hT[:, c0 : c0 + w], ps[:, :w], act.Relu)

        # ============ layer 2 ============
        # pass A: lo2 stream (gathers from hfirst) -> partial sums in meanhT
        l2lo_tiles = {}

        def l2lo_prefetch(b):
            for bp in range(b, min(b + AHEAD, nblk)):
                if bp not in l2lo_tiles:
                    l2lo_tiles[bp] = issue_gather(2, bp, hfirst[:])

        for b in range(nblk):
            l2lo_prefetch(b)
            c0 = b * P
            bs = min(P, rows_per - c0)
            ps = psum.tile([P, 512], dt.float32, tag="ps", name="ps_a2")
            gt, ot = l2lo_tiles.pop(b)
            for j in range(Ls[2]):
                nc.tensor.matmul(ps[:, :P], lhsT=gt[:, j, :], rhs=ot[:, j, :],
                                 start=(j == 0), stop=(j == Ls[2] - 1))
            nc.vector.tensor_copy(meanhT[:, c0 : c0 + bs], ps[:, :bs])

            if b == 17 and "collective" not in debug_skip:
                nc.gpsimd.collective_compute(
                    "AllGather", alu.bypass,
                    replica_groups=[list(range(m["n_cores"]))],
                    ins=[hshB[:].opt()], outs=[hsecond[:].opt()],
                )

        # pass B: hi2 stream (gathers from hsecond) += , then output dense
        l2hi_tiles = {}

        def l2hi_prefetch(b):
            for bp in range(b, min(b + AHEAD, nblk)):
                if bp not in l2hi_tiles:
                    l2hi_tiles[bp] = issue_gather(3, bp, hsecond[:])

        for b in range(nblk):
            l2hi_prefetch(b)
            c0 = b * P
            bs = min(P, rows_per - c0)
            ps = psum.tile([P, 512], dt.float32, tag="ps", name="ps_b2")
            gt, ot = l2hi_tiles.pop(b)
            for j in range(Ls[3]):
                nc.tensor.matmul(ps[:, :P], lhsT=gt[:, j, :], rhs=ot[:, j, :],
                                 start=(j == 0), stop=(j == Ls[3] - 1))
            nc.vector.tensor_tensor(meanhT[:, c0 : c0 + bs],
                                    meanhT[:, c0 : c0 + bs], ps[:, :bs], alu.add)

            ps2 = psum.tile([P, 512], dt.float32, tag="ps", name="ps_o")
            nc.tensor.matmul(ps2[:bs, :d_out], lhsT=meanhT[:, c0 : c0 + bs],
                             rhs=w2l_sb[:], start=True, stop=False)
            nc.tensor.matmul(ps2[:bs, :d_out], lhsT=hT[:, c0 : c0 + bs],
                             rhs=w2r_sb[:], start=False, stop=False)
            nc.tensor.matmul(ps2[:bs, :d_out], lhsT=ones_sb[:, :bs], rhs=b2_sb[:],
                             start=False, stop=True)
            otile = st_p.tile([P, d_out], dt.float32, tag="ot", name="ot")
            nc.vector.tensor_copy(otile[:bs, :], ps2[:bs, :d_out])
            nc.scalar.dma_start(out_d.ap()[c0 : c0 + bs, :], otile[:bs, :])

    return nc


def make_in_maps(inputs, meta, per_core):
    x = np.asarray(inputs["x"], np.float32)
    n_cores, rows_per = meta["n_cores"], meta["rows_per"]
    x_bf = x.astype(BF16)
    w1l = np.asarray(inputs["W1l"], np.float32)
    w1r = np.asarray(inputs["W1r"], np.float32)
    w2l = np.asarray(inputs["W2l"], np.float32)
    w2r = np.asarray(inputs["W2r"], np.float32)
    b1 = np.asarray(inputs["b1"], np.float32)
    b2 = np.asarray(inputs["b2"], np.float32)
    in_maps = []
    for k in range(n_cores):
        r0 = k * rows_per
        pc = per_core[k]
        im = {
            "x_all": x_bf,
            "xT": np.ascontiguousarray(x[r0 : r0 + rows_per].T).astype(BF16),
            "w1lT": np.ascontiguousarray(w1l.T).astype(BF16),
            "w1rT": np.ascontiguousarray(w1r.T).astype(BF16),
            "w2lT": np.ascontiguousarray(w2l.T).astype(BF16),
            "w2rT": np.ascontiguousarray(w2r.T).astype(BF16),
            "b1r": b1[None, :].astype(BF16),
            "b2r": b2[None, :].astype(BF16),
            "cnt": pc["cnt"],
        }
        for t in range(4):
            im[f"idx{t}"] = pc["idx"][t]
            im[f"oh{t}"] = pc["oh"][t]
        in_maps.append(im)
    return in_maps


_CACHE = {}


def _compile(meta):
    key = (meta["Ls"], meta["n_nodes"], meta["rows_per"])
    if key not in _CACHE:
        nc = bacc.Bacc("TRN2", target_bir_lowering=False, debug=False,
                       num_devices=meta["n_cores"], num_swdge_queues=4)
        build_graph(nc, meta)
        nc.compile()
        _CACHE[key] = nc
    return _CACHE[key]


def kernel(**inputs):
    edge_index = np.asarray(inputs["edge_index"])
    meta, per_core = preprocess(edge_index)
    nc = _compile(meta)
    in_maps = make_in_maps(inputs, meta, per_core)
    res = bass_utils.run_bass_kernel_spmd(
        nc, in_maps, core_ids=list(range(meta["n_cores"]))
    )
    out = np.concatenate(
        [res.results[k]["out"] for k in range(meta["n_cores"])], axis=0
    )
    return out.astype(np.float32)
